# revision 54
# baseline (speedup 1.0000x reference)
"""BERT self-attention on 8 Trainium2 NeuronCores.

Sharding: data-parallel over batch (4 cores per batch element) x
tensor-parallel over heads (4 heads per core). Q/K/V projections are
column-sharded, the output projection is row-sharded; each core returns a
partial [S, D] output which the host sums (+ b_o).

Per-core pipeline (batch b, 4 heads as 2 head-pairs mt, 256-col slice):
  The kernel is software-pipelined around the exp() stream on the
  Activation engine (131072 elems/partition at 1 elem/cycle ~ 109us) and
  the PE matmul stream (~210us of column-streams) -- PE-bound overall:

  - mt-outer, qb, then 8 rounds of kt-pairs per (mt, qb) block. Each
    round: scores(head A) -> expA -> [PV drain + filler] -> scores(head
    B) -> expB -> [PV drain + filler], so each head's scores execute in
    the shadow of the other head's exp and the PE never waits long.
  - K^T tiles are zero-padded to the full 128 contraction rows: the PE
    clock gate (HAM) only unthrottles for full-row matmuls (64-row
    tile_position variants measured ~274ns/MM vs ~250 padded).
  - PV keeps M=65: [V_h | 1] with the mask folded in multiplicatively;
    row 64 accumulates the softmax denominator for free.
  - Q/K/V projections and the output projection are emitted as deadline-
    tagged "filler" PE work between rounds (force-drained via need() so
    producers always precede consumers in emission order), letting ACT
    start ~8us in and the projections hide inside the attention stream.
  - Evict/normalize ride inside the last PV drain of each block and the
    pends queue carries across block boundaries, overlapping boundary
    work with the next block's scores. normalize: O^T *= 1/den via a
    stride-0 broadcast DMA read of the denominator row from DRAM.

Matmuls accumulate in fp32 PSUM; QK/QT and V'/expS run in bf16, the output
projection in float32r (full rate at N=512). PSUM budget: stageA(2) +
stageB(2) + op(2) + fill(2) = 8 banks. (fp8 DoubleRow PV was tried: ~25us
faster on paper but 2.1% rel err vs the 2e-2 gate -- ex and V quantization
each contribute ~1.4%.)
"""

import sys

for _p in ("/root/.axon_site/_ro/trn_rl_repo", "/opt/trn_rl_repo"):
    if _p not in sys.path:
        sys.path.append(_p)

import numpy as np
import ml_dtypes

BF16 = ml_dtypes.bfloat16

B, S, D, H, DH = 2, 2048, 1024, 16, 64
P = 128
NCORES = 8
HPC = 4              # heads per core
CW = HPC * DH        # 256: per-core feature slice width
DK = D // P          # 8 k-tiles over the model dim
SP = S // P          # 16 s-tiles
NB = 4               # 512-wide blocks over S
NW = S // NB         # 512
G = 2                # kt-pair group size (PSUM banks per stage tile)
NMT = HPC // 2       # 2 head pairs per core

_STATE = {}


def _build_nc():
    import concourse.bacc as bacc
    import concourse.tile as tile
    from concourse import mybir

    f32 = mybir.dt.float32
    f32r = mybir.dt.float32r
    bf16 = mybir.dt.bfloat16
    Exp = mybir.ActivationFunctionType.Exp

    nc = bacc.Bacc(None, target_bir_lowering=False, debug=False)

    with tile.TileContext(nc) as tc:
        with tc.tile_pool(name="dram", bufs=1, space="DRAM") as dram:
            xt = dram.tile([D, S], bf16, kind="ExternalInput", name="xt", uniquify=False)
            wq = dram.tile([P, DK, CW], bf16, kind="ExternalInput", name="wq", uniquify=False)
            wk = dram.tile([P, DK, CW], bf16, kind="ExternalInput", name="wk", uniquify=False)
            wv = dram.tile([P, DK, CW], bf16, kind="ExternalInput", name="wv", uniquify=False)
            wo = dram.tile([P, CW // P, D], bf16, kind="ExternalInput", name="wo", uniquify=False)
            bq = dram.tile([P, CW // P], f32, kind="ExternalInput", name="bq", uniquify=False)
            bk = dram.tile([P, CW // P], f32, kind="ExternalInput", name="bk", uniquify=False)
            bv = dram.tile([1, CW], f32, kind="ExternalInput", name="bv", uniquify=False)
            vmask = dram.tile([P, SP, HPC], f32, kind="ExternalInput", name="vmask", uniquify=False)
            y = dram.tile([S, D], f32, kind="ExternalOutput", name="y", uniquify=False)
            dden = dram.tile([HPC * NB, NW], bf16, name="dden")

            import concourse.bass as bass

            consts_cm = tc.tile_pool(name="consts", bufs=1)
            consts = consts_cm.__enter__()
            xt_sb = consts.tile([P, DK, S], bf16, name="xt_sb")
            wq_sb = consts.tile([P, DK, CW], bf16, name="wq_sb")
            wk_sb = consts.tile([P, DK, CW], bf16, name="wk_sb")
            wv_sb = consts.tile([P, DK, CW], bf16, name="wv_sb")
            wo_sb = consts.tile([P, CW // P, D], bf16, name="wo_sb")
            bq_sb = consts.tile([P, CW // P], f32, name="bq_sb")
            bk_sb = consts.tile([P, CW // P], f32, name="bk_sb")
            vbias_bc = consts.tile([P, CW], f32, name="vbias_bc")
            vmask_sb = consts.tile([P, SP, HPC], f32, name="vmask_sb")
            qt_sb = consts.tile([P, CW // P, S], bf16, name="qt_sb")
            # K^T per head, zero-padded to 128 contraction rows (the PE
            # clock gate only unthrottles for full-row matmuls)
            kz_sb = consts.tile([P, HPC, SP, P], bf16, name="kz_sb")
            zsrc = consts.tile([P, NW], bf16, name="zsrc")
            vp_sb = consts.tile([P, SP, HPC, DH + 1], bf16, name="vp_sb")
            ot_sb = consts.tile([P, CW // P, S], bf16, name="ot_sb")

            # ---- input DMAs, ordered so the prefix-critical tensors
            # (wq, X qb0 columns, wk, wv) arrive first ----
            nc.vector.memset(zsrc[:], 0.0)
            nc.sync.dma_start(out=wq_sb[:], in_=wq[:])
            nc.sync.dma_start(out=bq_sb[:], in_=bq[:])
            for k in range(DK):
                nc.sync.dma_start(out=xt_sb[:, k, 0:NW],
                                  in_=xt[k * P:(k + 1) * P, 0:NW])
            nc.sync.dma_start(out=wk_sb[:], in_=wk[:])
            nc.sync.dma_start(out=bk_sb[:], in_=bk[:])
            nc.sync.dma_start(out=wv_sb[:], in_=wv[:])
            nc.sync.dma_start(out=vmask_sb[:], in_=vmask[:])
            # broadcast b_v row across all 128 partitions (stride-0 DMA read)
            bv_row = bv[0:1, :]
            bv_bcast = bass.AP(
                tensor=bv_row.tensor,
                offset=bv_row.offset,
                ap=[[0, P]] + list(bv_row.ap[1:]),
            )
            nc.sync.dma_start(out=vbias_bc[:], in_=bv_bcast)
            for nb in range(1, NB):
                for k in range(DK):
                    nc.sync.dma_start(
                        out=xt_sb[:, k, nb * NW:(nb + 1) * NW],
                        in_=xt[k * P:(k + 1) * P, nb * NW:(nb + 1) * NW])
            nc.sync.dma_start(out=wo_sb[:], in_=wo[:])

            sb_pools_cm = [
                tc.tile_pool(name="exps_pool", bufs=4),
                tc.tile_pool(name="st65_pool", bufs=6),
                tc.tile_pool(name="bcast_pool", bufs=3),
                tc.tile_pool(name="y_pool", bufs=6),
            ]
            exps_pool, st65_pool, bcast_pool, y_pool = [c.__enter__() for c in sb_pools_cm]

            with tc.tile_pool(name="stageA_psum", bufs=1, space="PSUM") as stageA_psum, \
                 tc.tile_pool(name="stageB_psum", bufs=1, space="PSUM") as stageB_psum, \
                 tc.tile_pool(name="op_psum", bufs=2, space="PSUM") as op_psum, \
                 tc.tile_pool(name="fill_psum", bufs=2, space="PSUM") as fill_psum:

                # ---------- projection / output-proj emitters ----------
                def qproj_block(qb, m):
                    """Q^T block [128, 512] for (feature chunk m, q block qb)."""
                    qs = slice(qb * NW, (qb + 1) * NW)
                    psq = fill_psum.tile([P, NW], f32, name="fps")
                    def mms(ks):
                        def f():
                            for k in ks:
                                nc.tensor.matmul(
                                    psq[:], wq_sb[:, k, m * P:(m + 1) * P],
                                    xt_sb[:, k, qs],
                                    start=(k == 0), stop=(k == DK - 1))
                            if ks[-1] == DK - 1:
                                nc.vector.tensor_scalar_add(
                                    out=qt_sb[:, m, qs], in0=psq[:],
                                    scalar1=bq_sb[:, m:m + 1])
                        return f
                    return [mms(list(range(0, 4))), mms(list(range(4, DK)))]

                def kproj_block(m, nb):
                    """K^T block for head pair m, key block nb -> kz_sb."""
                    ns = slice(nb * NW, (nb + 1) * NW)
                    psk = fill_psum.tile([P, NW], f32, name="fps")
                    def mms(ks):
                        def f():
                            for k in ks:
                                nc.tensor.matmul(
                                    psk[:], wk_sb[:, k, m * P:(m + 1) * P],
                                    xt_sb[:, k, ns],
                                    start=(k == 0), stop=(k == DK - 1))
                            if ks[-1] == DK - 1:
                                nc.vector.tensor_scalar_add(
                                    out=kz_sb[0:DH, 2 * m, nb * 4:(nb + 1) * 4, :],
                                    in0=psk[0:DH, :].rearrange("p (a b) -> p a b", a=4),
                                    scalar1=bk_sb[0:DH, m:m + 1])
                                nc.vector.tensor_scalar_add(
                                    out=kz_sb[DH:P, 2 * m + 1, nb * 4:(nb + 1) * 4, :],
                                    in0=psk[DH:P, :].rearrange("p (a b) -> p a b", a=4),
                                    scalar1=bk_sb[DH:P, m:m + 1])
                                # zero the complementary contraction halves
                                # (keeps the 8.5us monolithic zero-fill off
                                # the critical path to the first exp)
                                nc.vector.tensor_copy(
                                    out=kz_sb[DH:P, 2 * m, nb * 4:(nb + 1) * 4, :],
                                    in_=zsrc[DH:P, :].rearrange("p (a b) -> p a b", a=4))
                                nc.vector.tensor_copy(
                                    out=kz_sb[0:DH, 2 * m + 1, nb * 4:(nb + 1) * 4, :],
                                    in_=zsrc[0:DH, :].rearrange("p (a b) -> p a b", a=4))
                        return f
                    return [mms(list(range(0, 4))), mms(list(range(4, DK)))]

                def vproj_block(st):
                    """V' rows for s-tile st: bias + mask folded, ones col."""
                    psv = fill_psum.tile([P, NW], f32, name="fps")
                    def mms(ks):
                        def f():
                            for k in ks:
                                nc.tensor.matmul(
                                    psv[:, 0:CW], xt_sb[:, k, st * P:(st + 1) * P],
                                    wv_sb[:, k, :],
                                    start=(k == 0), stop=(k == DK - 1))
                            if ks[-1] == DK - 1:
                                nc.vector.tensor_add(
                                    out=vp_sb[:, st, :, 0:DH],
                                    in0=psv[:, 0:CW].rearrange("p (h d) -> p h d", h=HPC),
                                    in1=vbias_bc[:].rearrange("p (h d) -> p h d", h=HPC))
                                nc.vector.tensor_copy(
                                    out=vp_sb[:, st, :, DH:DH + 1],
                                    in_=vmask_sb[:, st, :])
                                for h in range(HPC):
                                    nc.vector.tensor_scalar_mul(
                                        out=vp_sb[:, st, h, 0:DH],
                                        in0=vp_sb[:, st, h, 0:DH],
                                        scalar1=vmask_sb[:, st, h:h + 1])
                        return f
                    return [mms(list(range(0, 4))), mms(list(range(4, DK)))]

                def y_block(st, n2):
                    """Half an output-projection s-tile: Y[st, n2*512:...]."""
                    def f():
                        yps = fill_psum.tile([P, NW], f32, name="fps")
                        yt = y_pool.tile([P, NW], f32, name="yt")
                        for k2 in range(CW // P):
                            nc.tensor.matmul(
                                yps[:], ot_sb[:, k2, st * P:(st + 1) * P],
                                wo_sb[:, k2, n2 * NW:(n2 + 1) * NW],
                                start=(k2 == 0), stop=(k2 == CW // P - 1))
                        nc.vector.tensor_copy(out=yt[:], in_=yps[:])
                        nc.sync.dma_start(
                            out=y[st * P:(st + 1) * P, n2 * NW:(n2 + 1) * NW],
                            in_=yt[:])
                    return [f]

                # ---------- filler queue (deadline-ordered closures) ----------
                # Each entry is (tag, closure). Consumers force-drain via
                # need(tag) so a producer is always EMITTED before its
                # consumer (Tile only tracks read-after-write in emission
                # order); the queue order is a performance hint.
                from collections import deque
                fillers = deque()
                done_tags = set()

                def drain_one_filler():
                    tag, cl = fillers.popleft()
                    cl()
                    if tag is not None:
                        done_tags.add(tag)

                def drain_fillers(n):
                    for _ in range(n):
                        if not fillers:
                            return
                        drain_one_filler()

                def need(*tags):
                    for t in tags:
                        while t not in done_tags:
                            assert fillers, f"missing filler tag {t}"
                            drain_one_filler()

                def drain_all_fillers():
                    while fillers:
                        drain_one_filler()

                # ---------- attention helpers ----------
                def emit_evict(op_p, h_p, qb_p):
                    qs = slice(qb_p * NW, (qb_p + 1) * NW)
                    mt, po = h_p // 2, (h_p % 2) * DH
                    st65 = st65_pool.tile([DH + 1, NW], bf16, name="st65")
                    nc.vector.tensor_copy(out=st65[:], in_=op_p[:])
                    nc.sync.dma_start(
                        out=ot_sb[po:po + DH, mt, qs], in_=st65[0:DH, :])
                    nc.sync.dma_start(
                        out=dden[h_p * NB + qb_p, :], in_=st65[DH:DH + 1, :])

                def emit_normalize(qb_p, mt):
                    qs = slice(qb_p * NW, (qb_p + 1) * NW)
                    bc = bcast_pool.tile([P, NW], bf16, name="bc")
                    for half in range(2):
                        hh = 2 * mt + half
                        den_row = dden[hh * NB + qb_p:hh * NB + qb_p + 1, :]
                        den_bcast = bass.AP(
                            tensor=den_row.tensor,
                            offset=den_row.offset,
                            ap=[[0, DH]] + list(den_row.ap[1:]),
                        )
                        nc.sync.dma_start(
                            out=bc[half * DH:(half + 1) * DH, :], in_=den_bcast)
                    with nc.allow_low_precision(reason="softmax denom bf16; ~0.5% vs 2e-2 gate"):
                        nc.vector.reciprocal(out=bc[:], in_=bc[:])
                        nc.vector.tensor_mul(
                            out=ot_sb[:, mt, qs],
                            in0=ot_sb[:, mt, qs], in1=bc[:])

                pends = deque()   # (op_ps, h, kts, ex, last, qb)
                y_stash = deque()  # y closures delayed ~a round so their
                                   # MMs don't stall on the normalize chain

                def drain_pv():
                    # drain ONE pend (one head's kt pair). On the last
                    # round's odd-head entry also evict + normalize here,
                    # so carried-over pends overlap the next block's scores.
                    op_p, h_p, kts_p, ex_p, last, qb_p = pends.popleft()
                    for i, kt in enumerate(kts_p):
                        nc.tensor.matmul(
                            op_p[:], vp_sb[:, kt, h_p, :], ex_p[:, i, :],
                            start=(kt == 0), stop=(kt == SP - 1),
                            skip_group_check=True)
                    if last:
                        emit_evict(op_p, h_p, qb_p)
                        if h_p % 2 == 1:
                            mt_p = h_p // 2
                            emit_normalize(qb_p, mt_p)
                            if mt_p == 1:
                                for st in range(4 * qb_p, 4 * qb_p + 4):
                                    for n2 in range(2):
                                        for cl in y_block(st, n2):
                                            y_stash.append(cl)

                def need_pv_inputs():
                    if pends:
                        nkt = pends[0][2][-1]
                        need(("vp", nkt - 1), ("vp", nkt))

                # ---------- prefix: minimum work before round 0 ----------
                for cl in qproj_block(0, 0) + qproj_block(0, 1):
                    cl()
                for cl in kproj_block(0, 0):
                    cl()
                for cl in vproj_block(0) + vproj_block(1):
                    cl()
                done_tags.update({("qp", 0, 0), ("qp", 0, 1), ("kp", 0, 0),
                                  ("vp", 0), ("vp", 1)})

                # filler order ~ by deadline: mt0's V/K inside the first qb
                # sweep, Q blocks next, then mt1's K/V, then mt1's Q
                def enq(tag, cls):
                    for cl in cls[:-1]:
                        fillers.append((None, cl))
                    fillers.append((tag, cls[-1]))

                enq(("kp", 0, 1), kproj_block(0, 1))
                enq(("vp", 2), vproj_block(2))
                enq(("vp", 3), vproj_block(3))
                enq(("kp", 0, 2), kproj_block(0, 2))
                enq(("vp", 4), vproj_block(4))
                enq(("vp", 5), vproj_block(5))
                enq(("kp", 0, 3), kproj_block(0, 3))
                for st in range(6, SP):
                    enq(("vp", st), vproj_block(st))
                for qb in range(1, NB):
                    enq(("qp", qb, 0), qproj_block(qb, 0))
                for nb in range(NB):
                    enq(("kp", 1, nb), kproj_block(1, nb))
                for qb in range(1, NB):
                    enq(("qp", qb, 1), qproj_block(qb, 1))

                # ---------- main loop: mt outer, qb, kt-pair rounds ----------
                for mt in range(NMT):
                    hA, hB = 2 * mt, 2 * mt + 1
                    for qb in range(NB):
                        qs = slice(qb * NW, (qb + 1) * NW)
                        need(("qp", qb, mt))
                        opA = op_psum.tile([DH + 1, NW], f32, name="op_ps")
                        opB = op_psum.tile([DH + 1, NW], f32, name="op_ps")
                        for r in range(SP // G):
                            kts = [G * r, G * r + 1]
                            need(("kp", mt, kts[0] // 4), ("kp", mt, kts[-1] // 4))
                            need_pv_inputs()
                            if r >= 2:
                                for _ in range(2):
                                    if y_stash:
                                        fillers.append((None, y_stash.popleft()))
                            # A block fully before B block: A's scores run
                            # during expB(r-1), B's during expA(r) -- an
                            # interleaved order would stall A's stream on
                            # the B-side semaphore
                            stA = stageA_psum.tile([P, G, NW], f32, name="st_a")
                            for i, kt in enumerate(kts):
                                nc.tensor.matmul(
                                    stA[:, i, :], kz_sb[:, hA, kt, :],
                                    qt_sb[:, mt, qs],
                                    start=True, stop=True)
                            exA = exps_pool.tile([P, G, NW], bf16, name="ex")
                            nc.scalar.activation(
                                out=exA[:], in_=stA[:],
                                func=Exp, scale=1.0 / np.sqrt(DH))
                            if pends:
                                drain_pv()
                            drain_fillers(1)
                            stB = stageB_psum.tile([P, G, NW], f32, name="st_b")
                            for i, kt in enumerate(kts):
                                nc.tensor.matmul(
                                    stB[:, i, :], kz_sb[:, hB, kt, :],
                                    qt_sb[:, mt, qs],
                                    start=True, stop=True)
                            exB = exps_pool.tile([P, G, NW], bf16, name="ex")
                            nc.scalar.activation(
                                out=exB[:], in_=stB[:],
                                func=Exp, scale=1.0 / np.sqrt(DH))
                            if pends:
                                drain_pv()
                            drain_fillers(1)
                            last = (r == SP // G - 1)
                            pends.append((opA, hA, kts, exA, last, qb))
                            pends.append((opB, hB, kts, exB, last, qb))

                while pends:
                    need_pv_inputs()
                    drain_pv()
                while y_stash:
                    fillers.append((None, y_stash.popleft()))
                drain_all_fillers()

            for c in reversed(sb_pools_cm):
                c.__exit__(None, None, None)
            consts_cm.__exit__(None, None, None)

    nc.compile()
    return nc


def _get_nc():
    if "nc" not in _STATE:
        _STATE["nc"] = _build_nc()
    return _STATE["nc"]


def _make_in_maps(hidden_states, attention_mask, W_q, b_q, W_k, b_k, W_v, b_v, W_o):
    hs = np.asarray(hidden_states, dtype=np.float32)
    mask = np.asarray(attention_mask)
    W_q = np.asarray(W_q, dtype=np.float32)
    W_k = np.asarray(W_k, dtype=np.float32)
    W_v = np.asarray(W_v, dtype=np.float32)
    W_o = np.asarray(W_o, dtype=np.float32)
    b_q = np.asarray(b_q, dtype=np.float32)
    b_k = np.asarray(b_k, dtype=np.float32)
    b_v = np.asarray(b_v, dtype=np.float32)

    in_maps = []
    for c in range(NCORES):
        b, j = c // (NCORES // B), c % (NCORES // B)
        cols = slice(CW * j, CW * (j + 1))
        xt = np.ascontiguousarray(hs[b].T.astype(BF16))                      # [D, S]
        wq_c = np.ascontiguousarray(W_q[:, cols].reshape(DK, P, CW).transpose(1, 0, 2).astype(BF16))
        wk_c = np.ascontiguousarray(W_k[:, cols].reshape(DK, P, CW).transpose(1, 0, 2).astype(BF16))
        wv_c = np.ascontiguousarray(W_v[:, cols].reshape(DK, P, CW).transpose(1, 0, 2).astype(BF16))
        wo_c = np.ascontiguousarray(W_o[cols, :].reshape(CW // P, P, D).transpose(1, 0, 2).astype(BF16))
        bqc = np.ascontiguousarray(b_q[cols].reshape(CW // P, P).T)          # [128, 2]
        bkc = np.ascontiguousarray(b_k[cols].reshape(CW // P, P).T)
        bvc = np.ascontiguousarray(b_v[cols].reshape(1, CW))
        m = mask[b * H + HPC * j: b * H + HPC * (j + 1), 0, :].astype(np.float32)  # [4, S]
        vm = np.ascontiguousarray(m.reshape(HPC, SP, P).transpose(2, 1, 0))  # [128, 16, 4]
        in_maps.append({
            "xt": xt, "wq": wq_c, "wk": wk_c, "wv": wv_c, "wo": wo_c,
            "bq": bqc, "bk": bkc, "bv": bvc, "vmask": vm,
        })
    return in_maps


def run(inputs, trace=False, **trace_kwargs):
    """Run the SPMD kernel. Returns (full_output, BassKernelResults)."""
    from concourse.bass_utils import run_bass_kernel_spmd

    nc = _get_nc()
    in_maps = _make_in_maps(
        inputs["hidden_states"], inputs["attention_mask"],
        inputs["W_q"], inputs["b_q"], inputs["W_k"], inputs["b_k"],
        inputs["W_v"], inputs["b_v"], inputs["W_o"])
    res = run_bass_kernel_spmd(
        nc, in_maps, list(range(NCORES)), trace=trace, **trace_kwargs)

    b_o = np.asarray(inputs["b_o"], dtype=np.float32)
    out = np.zeros((B, S, D), dtype=np.float32)
    gpb = NCORES // B
    for c in range(NCORES):
        out[c // gpb] += res.results[c]["y"]
    out += b_o[None, None, :]
    return out, res


def kernel(**inputs):
    out, _ = run(inputs, trace=False)
    return out
